# revision 1
# baseline (speedup 1.0000x reference)
"""Hausdorff distance kernel for Trainium2 (8 NeuronCores).

Reference computes, per sample n (N=2), on a 20^3 voxel grid (V=8000):
  d[i,j]   = Euclidean distance between voxel centers (coords / 20)
  min_to_B = min over j in B of d[i,j]
  distA    = max over i in Aonly of min_to_B   (Aonly = A & ~B)
  (symmetrically distB), haus_n = max(distA, distB); output = mean_n haus_n.

Strategy:
 - Host compacts each (sample, direction) to a KNN problem: rows = Aonly
   points (~V/4), cols = B points (~V/2). 4 problems total; core c handles
   (problem c//2, row-half c%2).
 - Distances are computed in *unnormalized integer* coordinates: d2 =
   dx^2+dy^2+dz^2 <= 3*19^2 = 1083. All matmul inputs are small integers,
   exactly representable in fp16; PSUM accumulates in fp32 => d2 is exact.
   sqrt(d2/400) is applied on host (monotone => same argmin/argmax).
 - On-chip: d2 tile [128,512] via one K=5 matmul (lhsT=[sq_i,1,-2x,-2y,-2z],
   rhs=[1,sq_j,x,y,z]); row-min via DVE tensor_reduce; final max on host.
"""

import sys
import functools

import numpy as np

for _p in ("/opt/trn_rl_repo",):
    if _p not in sys.path:
        sys.path.insert(0, _p)

from concourse import bass, mybir, tile  # noqa: E402
from concourse.bass_utils import run_bass_kernel_spmd  # noqa: E402

D = H = W = 20
V = D * H * W
N_CORES = 8
BIG = 1e9
F16 = mybir.dt.float16
F32 = mybir.dt.float32


def _coords_int():
    x, y, z = np.meshgrid(np.arange(D), np.arange(H), np.arange(W), indexing="ij")
    return np.stack([x, y, z], axis=-1).reshape(V, 3).astype(np.float64)


_COORDS = _coords_int()


def _round_up(x, m):
    return max(m, ((int(x) + m - 1) // m) * m)


@functools.lru_cache(maxsize=None)
def _build(r_cap, c_cap):
    """Raw-bass program: per-core [5,r_cap] x [5,c_cap] -> row mins [128, r_cap//128].

    Hand-rolled semaphores (Tile's tail Drain exceeds the ISA's sync-wait
    slot limit for this engine mix, and its end-of-kernel barrier costs
    ~10us anyway). Pipeline: PE fills PSUM banks round-robin, DVE min-
    reduces each bank, PE reuses a bank only after its reduce completed.
    """
    nrt = r_cap // 128
    nct = c_cap // 512
    nc = bass.Bass()
    # single input tensor: [5, r_cap] lhsT block then [5, c_cap] rhs block
    inp_d = nc.declare_dram_parameter("inp", [5, r_cap + c_cap], F16, isOutput=False)
    out_d = nc.declare_dram_parameter("out", [128, nrt], F32, isOutput=True)

    with (
        nc.sbuf_tensor("inp_t", [5, r_cap + c_cap], F16) as inp_t,
        nc.sbuf_tensor("strip", [128, nrt, nct], F32) as strip,
        nc.sbuf_tensor("allmin", [128, nrt], F32) as allmin,
        nc.psum_tensor("ps", [128, 8, 512], F32) as ps,
        nc.semaphore("in_sem") as in_sem,
        nc.semaphore("pe_sem") as pe_sem,
        nc.semaphore("dve_sem") as dve_sem,
        nc.semaphore("out_sem") as out_sem,
    ):
        lhsT_t = inp_t.ap()[:, :r_cap]
        rhs_t = inp_t.ap()[:, r_cap:]

        nc.sync.dma_start(out=inp_t.ap(), in_=inp_d[:]).then_inc(in_sem, 16)

        # PE program: matmul k -> psum bank k%8; reuse bank after DVE drained it.
        # DVE instruction order: per rt, nct strip-reduces then 1 final reduce,
        # each bumping dve_sem by 1 => strip-reduce k completes at
        # dve_sem == k + k//nct + 1.
        nc.tensor.wait_ge(in_sem, 16)
        k = 0
        for rt in range(nrt):
            for ct in range(nct):
                if k >= 8:
                    kd = k - 8
                    nc.tensor.wait_ge(dve_sem, kd + kd // nct + 1)
                nc.tensor.matmul(
                    ps.ap()[:, k % 8, :],
                    lhsT_t[:, rt * 128 : (rt + 1) * 128],
                    rhs_t[:, ct * 512 : (ct + 1) * 512],
                ).then_inc(pe_sem, 1)
                k += 1

        # DVE program
        k = 0
        for rt in range(nrt):
            for ct in range(nct):
                nc.vector.wait_ge(pe_sem, k + 1)
                nc.vector.tensor_reduce(
                    strip.ap()[:, rt, ct : ct + 1],
                    ps.ap()[:, k % 8, :],
                    axis=mybir.AxisListType.X,
                    op=mybir.AluOpType.min,
                ).then_inc(dve_sem, 1)
                k += 1
            # same-engine RAW on strip: wait for this rt's strip writes to land
            nc.vector.wait_ge(dve_sem, rt * (nct + 1) + nct)
            nc.vector.tensor_reduce(
                allmin.ap()[:, rt : rt + 1],
                strip.ap()[:, rt, :],
                axis=mybir.AxisListType.X,
                op=mybir.AluOpType.min,
            ).then_inc(dve_sem, 1)

        # output DMA after every DVE op retired
        nc.sync.wait_ge(dve_sem, nrt * (nct + 1))
        nc.sync.dma_start(out=out_d[:], in_=allmin.ap()).then_inc(out_sem, 16)
        nc.sync.wait_ge(out_sem, 16)
    return nc


def _make_lhsT(pts, r_cap):
    """pts: [k,3] integer coords. Rows: [sq_i, 1, -2x, -2y, -2z], fp16-exact."""
    k = len(pts)
    if k == 0:
        pts = np.zeros((1, 3))
        k = 1
    pad = np.concatenate([pts, np.broadcast_to(pts[0], (r_cap - k, 3))], axis=0)
    arr = np.empty((5, r_cap), np.float16)
    arr[0] = (pad**2).sum(1)
    arr[1] = 1.0
    arr[2] = -2.0 * pad[:, 0]
    arr[3] = -2.0 * pad[:, 1]
    arr[4] = -2.0 * pad[:, 2]
    return arr


def _make_rhs(pts, c_cap):
    """pts: [m,3] integer coords. Rows: [1, sq_j, x, y, z], fp16-exact."""
    m = len(pts)
    if m == 0:
        pts = np.zeros((1, 3))
        m = 1
    pad = np.concatenate([pts, np.broadcast_to(pts[0], (c_cap - m, 3))], axis=0)
    arr = np.empty((5, c_cap), np.float16)
    arr[0] = 1.0
    arr[1] = (pad**2).sum(1)
    arr[2] = pad[:, 0]
    arr[3] = pad[:, 1]
    arr[4] = pad[:, 2]
    return arr


def kernel(predict, target):
    predict = np.asarray(predict)
    target = np.asarray(target)
    n = predict.shape[0]
    im_a = (np.round(predict.reshape(n, V)) != 0)
    im_b = (np.round(target.reshape(n, V)) != 0)

    # 2*n directed problems: (rows = one-sided points, cols = other full set)
    probs = []
    for s in range(n):
        ma, mb = im_a[s], im_b[s]
        probs.append((_COORDS[ma & ~mb], _COORDS[mb]))  # distA direction
        probs.append((_COORDS[mb & ~ma], _COORDS[ma]))  # distB direction
    n_probs = len(probs)
    halves = N_CORES // n_probs  # 2 for N=2

    # per-core row slices (contiguous split into `halves` chunks)
    core_rows = []
    for p in range(n_probs):
        rows = probs[p][0]
        k = len(rows)
        per = -(-max(k, 1) // halves)
        for h in range(halves):
            core_rows.append(rows[h * per : (h + 1) * per])

    r_cap = _round_up(max((len(r) for r in core_rows), default=1), 128)
    c_cap = _round_up(max((len(c) for _, c in probs), default=1), 512)

    nc = _build(r_cap, c_cap)
    in_maps = []
    for c in range(N_CORES):
        p = c // halves
        inp = np.concatenate(
            [_make_lhsT(core_rows[c], r_cap), _make_rhs(probs[p][1], c_cap)], axis=1
        )
        in_maps.append({"inp": inp})
    results = run_bass_kernel_spmd(nc, in_maps, list(range(N_CORES))).results

    # out[i, rt] = min d2 for row rt*128+i  -> flatten to [r_cap]
    core_mins = [np.asarray(results[c]["out"]).T.reshape(-1) for c in range(N_CORES)]

    dists = np.empty(n_probs, np.float64)
    for p in range(n_probs):
        rows, cols = probs[p]
        if len(rows) == 0:
            dists[p] = 0.0
            continue
        if len(cols) == 0:
            # reference: min_to_X == BIG everywhere -> directed dist = BIG;
            # (distB's 999.0 special case is applied below)
            dists[p] = BIG
            continue
        parts = []
        for h in range(halves):
            cr = core_rows[p * halves + h]
            if len(cr):
                parts.append(core_mins[p * halves + h][: len(cr)])
        d2max = max(float(x.max()) for x in parts)
        dists[p] = np.sqrt(d2max / 400.0)

    haus = np.empty(n, np.float64)
    for s in range(n):
        dist_a, dist_b = dists[2 * s], dists[2 * s + 1]
        ma, mb = im_a[s], im_b[s]
        if (mb & ~ma).any() and not ma.any():
            dist_b = 999.0
        haus[s] = max(dist_a, dist_b)
    return np.float32(haus.mean())



# revision 21
# speedup vs baseline: 2.7867x; 2.7867x over previous
"""Hausdorff distance kernel for Trainium2 (8 NeuronCores).

Reference computes, per sample n (N=2), on a 20^3 voxel grid (V=8000):
  d[i,j]   = Euclidean distance between voxel centers (coords / 20)
  min_to_B = min over j in B of d[i,j]
  distA    = max over i in Aonly of min_to_B   (Aonly = A & ~B)
  (symmetrically distB), haus_n = max(distA, distB); output = mean_n haus_n.

Strategy (separable Gaussian-sum distance field):
  On the integer grid, md2[p] = min_{q in B} |p-q|^2 is an integer. With
  S[p] = sum_{q in B} exp(-a*|p-q|^2) and a=10, we get
      -ln(S[p])/a = md2[p] - ln(n0 + eps)/a,   n0 = #minimizers <= 24,
  so round(-ln(S)/a) recovers md2 EXACTLY while S stays in fp32 range
  (md2 <= 8; larger values underflow detectably -> exact host fallback,
  which cannot trigger unless the Hausdorff distance exceeds sqrt(8)
  voxels).  exp(-a*|p-q|^2) factors over axes, so S = (Kx x Ky x Kz) * B
  is computed with 5 small matmuls per (sample, direction):
    stage 1:  out1[x,(y',z')] = sum_{(y,z)} B[(y,z),x] * K2[(y,z),(y',z')]
              (4 PSUM-accumulated matmuls, contraction 400 split 4x100)
    stage 2:  S[x',(y',z')]   = sum_x Kx[x,x'] * out1[x,(y',z')]
  then min over the Aonly mask via one DVE tensor_tensor_reduce and a
  GpSimd partition-min to a single scalar (1-packet output DMA keeps the
  DMA-completion tail short).  8 cores = 4 (sample,direction) problems
  x 2 halves of the (y',z') output space.
"""

import sys
import functools

import numpy as np

for _p in ("/opt/trn_rl_repo",):
    if _p not in sys.path:
        sys.path.insert(0, _p)

import ml_dtypes  # noqa: E402
from concourse import bass, mybir  # noqa: E402
from concourse.bass_utils import run_bass_kernel_spmd  # noqa: E402

D = H = W = 20
V = D * H * W
N_CORES = 8
BIG = 1e9
ALPHA = 10.0
S_FLOOR = 1e-36  # S below this => md2 >= 9 possible => exact fallback
MASK_BIG = 3.0e38
F32 = mybir.dt.float32
BF16 = mybir.dt.bfloat16
NPBF16 = ml_dtypes.bfloat16

# kin layout (bf16, [100, 900]):
#   [:, 0:80)      B-mask chunks: chunk c at cols [20c, 20c+20), rows (y,z) 100c..
#   [:, 80:880)    K2 chunks: chunk c at cols [80+200c, 280+200c)
#   [0:20, 880:900) Kx
# msk (fp32, [20, 200]): 0.0 / MASK_BIG row mask for this (y',z') half
KIN_COLS = 900


@functools.lru_cache(maxsize=None)
def _kernels():
    yy, zz = np.meshgrid(np.arange(H), np.arange(W), indexing="ij")
    yz = np.stack([yy, zz], -1).reshape(400, 2).astype(np.float64)
    k2 = np.exp(-ALPHA * ((yz[:, None, :] - yz[None, :, :]) ** 2).sum(-1))
    kx = np.exp(-ALPHA * (np.subtract.outer(np.arange(D), np.arange(D)) ** 2.0))
    return k2.astype(NPBF16), kx.astype(NPBF16)


@functools.lru_cache(maxsize=None)
def _build():
    """Per-core program: S-field for one (problem, half) -> masked min scalar."""
    nc = bass.Bass()
    kin_d = nc.declare_dram_parameter("kin", [100, KIN_COLS], BF16, isOutput=False)
    msk_d = nc.declare_dram_parameter("msk", [20, 200], F32, isOutput=False)
    out_d = nc.declare_dram_parameter("out", [1, 32], F32, isOutput=True)

    with (
        nc.sbuf_tensor("kin_t", [100, KIN_COLS], BF16) as kin_t,
        nc.sbuf_tensor("msk_t", [20, 200], F32) as msk_t,
        nc.sbuf_tensor("out1", [20, 200], BF16) as out1,
        nc.sbuf_tensor("scratch", [20, 200], F32) as scratch,
        nc.sbuf_tensor("red32", [32, 32], F32) as red32,
        nc.sbuf_tensor("fin32", [32, 32], F32) as fin32,
        nc.psum_tensor("ps1", [20, 200], F32) as ps1,
        nc.psum_tensor("ps2", [20, 200], F32) as ps2,
        nc.semaphore("in_sem") as in_sem,
        nc.semaphore("pe_sem") as pe_sem,
        nc.semaphore("sc_sem") as sc_sem,
        nc.semaphore("dve_sem") as dve_sem,
        nc.semaphore("out_sem") as out_sem,
    ):
        nc.sync.dma_start(out=kin_t.ap(), in_=kin_d[:]).then_inc(in_sem, 16)
        nc.sync.dma_start(out=msk_t.ap(), in_=msk_d[:]).then_inc(in_sem, 16)

        # stage 1: 4 accumulating matmuls over (y,z) chunks of 100
        nc.tensor.wait_ge(in_sem, 16)
        for c in range(4):
            mm = nc.tensor.matmul(
                ps1.ap(),
                kin_t.ap()[:, 20 * c : 20 * (c + 1)],
                kin_t.ap()[:, 80 + 200 * c : 280 + 200 * c],
                start=(c == 0),
                stop=(c == 3),
            )
        mm.then_inc(pe_sem, 1)

        # PSUM -> SBUF bf16 for stage-2 moving operand
        nc.scalar.wait_ge(pe_sem, 1)
        nc.scalar.copy(out1.ap(), ps1.ap()).then_inc(sc_sem, 1)

        # stage 2: contract x with Kx
        nc.tensor.wait_ge(sc_sem, 1)
        nc.tensor.matmul(
            ps2.ap(), kin_t.ap()[0:20, 880:900], out1.ap(), start=True, stop=True
        ).then_inc(pe_sem, 1)

        # fill the transpose tile with +inf-ish so unused lanes lose the min
        nc.vector.memset(red32.ap(), 3.4e38)

        # masked min over free dim: red[x] = min_j (S[x,j] + mask[x,j])
        # (same-engine RAW chains: DVE is pipelined, needs self-waits)
        nc.vector.wait_ge(pe_sem, 2)
        nc.vector.tensor_tensor(
            scratch.ap(), ps2.ap(), msk_t.ap(), op=mybir.AluOpType.add
        ).then_inc(dve_sem, 1)
        nc.vector.wait_ge(dve_sem, 1)
        nc.vector.tensor_reduce(
            red32.ap()[0:20, 0:1],
            scratch.ap(),
            axis=mybir.AxisListType.X,
            op=mybir.AluOpType.min,
        ).then_inc(dve_sem, 1)

        # 32x32 stream transpose puts the 20 per-x minima on partition 0
        nc.vector.wait_ge(dve_sem, 2)
        nc.vector.transpose(fin32.ap(), red32.ap()).then_inc(dve_sem, 1)

        nc.sync.wait_ge(dve_sem, 3)
        nc.sync.dma_start(
            out=out_d[:], in_=fin32.ap()[0:1, 0:32], single_packet=True
        ).then_inc(out_sem, 16)
        nc.sync.wait_ge(out_sem, 16)
    return nc


def _make_kin(rows_mask, cols_mask, half):
    """Build the bf16 kin + fp32 msk inputs for one (problem, half)."""
    k2, kx = _kernels()
    b3 = cols_mask.reshape(D, H, W)
    byz_x = np.ascontiguousarray(b3.transpose(1, 2, 0).reshape(400, D)).astype(NPBF16)
    rows3 = rows_mask.reshape(D, H, W)
    maskbig = np.where(
        rows3[:, 10 * half : 10 * (half + 1), :].reshape(D, 200), 0.0, MASK_BIG
    ).astype(np.float32)

    kin = np.zeros((100, KIN_COLS), NPBF16)
    for c in range(4):
        kin[:, 20 * c : 20 * (c + 1)] = byz_x[100 * c : 100 * (c + 1)]
        kin[:, 80 + 200 * c : 280 + 200 * c] = k2[
            100 * c : 100 * (c + 1), 200 * half : 200 * (half + 1)
        ]
    kin[0:20, 880:900] = kx
    return kin, maskbig


def _exact_md2max(rows_mask, cols_mask):
    """Host fallback: exact max-min squared distance (integer grid)."""
    x, y, z = np.meshgrid(np.arange(D), np.arange(H), np.arange(W), indexing="ij")
    coords = np.stack([x, y, z], -1).reshape(V, 3).astype(np.float64)
    rows = coords[rows_mask]
    cols = coords[cols_mask]
    best = 0.0
    for i in range(0, len(rows), 512):
        blk = rows[i : i + 512]
        d2 = ((blk[:, None, :] - cols[None, :, :]) ** 2).sum(-1)
        best = max(best, d2.min(axis=1).max())
    return best


def kernel(predict, target):
    predict = np.asarray(predict)
    target = np.asarray(target)
    n = predict.shape[0]
    im_a = np.round(predict.reshape(n, V)) != 0
    im_b = np.round(target.reshape(n, V)) != 0

    # 2*n directed problems: (rows = one-sided points, cols = other full set)
    probs = []
    for s in range(n):
        ma, mb = im_a[s], im_b[s]
        probs.append((ma & ~mb, mb))  # distA direction
        probs.append((mb & ~ma, ma))  # distB direction
    n_probs = len(probs)
    assert n_probs * 2 == N_CORES, "kernel hardcodes N=2 samples -> 4 problems"

    nc = _build()
    in_maps = []
    for c in range(N_CORES):
        rows_mask, cols_mask = probs[c // 2]
        if not rows_mask.any() or not cols_mask.any():
            # degenerate problem: feed a dummy; host overrides the result
            rows_mask = np.ones(V, bool)
            cols_mask = np.ones(V, bool)
        kin, msk = _make_kin(rows_mask, cols_mask, c % 2)
        in_maps.append({"kin": kin, "msk": msk})
    results = run_bass_kernel_spmd(nc, in_maps, list(range(N_CORES))).results

    dists = np.empty(n_probs, np.float64)
    for p in range(n_probs):
        rows_mask, cols_mask = probs[p]
        if not rows_mask.any():
            dists[p] = 0.0
            continue
        if not cols_mask.any():
            dists[p] = BIG  # reference: min over empty B stays BIG
            continue
        s_min = min(
            float(np.asarray(results[2 * p]["out"]).reshape(-1)[:20].min()),
            float(np.asarray(results[2 * p + 1]["out"]).reshape(-1)[:20].min()),
        )
        est = -np.log(max(s_min, 1e-45)) / ALPHA if s_min > 0 else np.inf
        if s_min < S_FLOOR or abs(est - np.round(est)) > 0.4:
            md2 = _exact_md2max(rows_mask, cols_mask)  # device range exceeded
        else:
            md2 = float(np.round(est))
        dists[p] = np.sqrt(md2) / 20.0

    haus = np.empty(n, np.float64)
    for s in range(n):
        dist_a, dist_b = dists[2 * s], dists[2 * s + 1]
        ma, mb = im_a[s], im_b[s]
        if (mb & ~ma).any() and not ma.any():
            dist_b = 999.0
        haus[s] = max(dist_a, dist_b)
    return np.float32(haus.mean())


# revision 25
# speedup vs baseline: 3.4597x; 1.2415x over previous
"""Hausdorff distance kernel for Trainium2 (8 NeuronCores).

Reference computes, per sample n (N=2), on a 20^3 voxel grid (V=8000):
  d[i,j]   = Euclidean distance between voxel centers (coords / 20)
  min_to_B = min over j in B of d[i,j]
  distA    = max over i in Aonly of min_to_B   (Aonly = A & ~B)
  (symmetrically distB), haus_n = max(distA, distB); output = mean_n haus_n.

Strategy (separable Gaussian-sum distance field):
  On the integer grid, md2[p] = min_{q in B} |p-q|^2 is an integer. With
  S[p] = sum_{q in B} exp(-a*|p-q|^2) and a=10, we get
      -ln(S[p])/a = md2[p] - ln(n0 + eps)/a,   n0 = #minimizers <= 24,
  so round(-ln(S)/a) recovers md2 EXACTLY while S stays in fp32 range
  (md2 <= 8; larger values underflow detectably -> exact host fallback,
  which cannot trigger unless the Hausdorff distance exceeds sqrt(8)
  voxels).  exp(-a*|p-q|^2) factors over axes, so S = (Kx x Ky x Kz) * B
  is computed with 5 small matmuls per (sample, direction):
    stage 1:  out1[x,(y',z')] = sum_{(y,z)} B[(y,z),x] * K2[(y,z),(y',z')]
              (4 PSUM-accumulated matmuls, contraction 400 split 4x100)
    stage 2:  S[x',(y',z')]   = sum_x Kx[x,x'] * out1[x,(y',z')]
  then min over the Aonly mask via one DVE tensor_tensor_reduce and a
  GpSimd partition-min to a single scalar (1-packet output DMA keeps the
  DMA-completion tail short).  8 cores = 4 (sample,direction) problems
  x 2 halves of the (y',z') output space.
"""

import sys
import functools

import numpy as np

for _p in ("/opt/trn_rl_repo",):
    if _p not in sys.path:
        sys.path.insert(0, _p)

import ml_dtypes  # noqa: E402
from concourse import bass, mybir  # noqa: E402
from concourse.bass_utils import run_bass_kernel_spmd  # noqa: E402

D = H = W = 20
V = D * H * W
N_CORES = 8
BIG = 1e9
ALPHA = 10.0
S_FLOOR = 1e-36  # S below this => md2 >= 9 possible => exact fallback
MASK_BIG = 3.0e38
F32 = mybir.dt.float32
BF16 = mybir.dt.bfloat16
NPBF16 = ml_dtypes.bfloat16

# kin layout (bf16, [100, 1120]):
#   [:, 0:80)       B-mask chunks: chunk c at cols [20c, 20c+20), rows (y,z) 100c..
#   [:, 80:880)     K2 chunks: chunk c at cols [80+200c, 280+200c)
#   [0:20, 880:900)  Kx
#   [0:20, 900:1100) row mask as 0.0 / MASK_BIG for this (y',z') half
#   [0:20, 1100:1120) identity (to matmul-accumulate the mask into PSUM)
KIN_COLS = 1120


@functools.lru_cache(maxsize=None)
def _kernels():
    yy, zz = np.meshgrid(np.arange(H), np.arange(W), indexing="ij")
    yz = np.stack([yy, zz], -1).reshape(400, 2).astype(np.float64)
    k2 = np.exp(-ALPHA * ((yz[:, None, :] - yz[None, :, :]) ** 2).sum(-1))
    kx = np.exp(-ALPHA * (np.subtract.outer(np.arange(D), np.arange(D)) ** 2.0))
    return k2.astype(NPBF16), kx.astype(NPBF16)


@functools.lru_cache(maxsize=None)
def _build():
    """Per-core program: S-field for one (problem, half) -> masked min scalar."""
    nc = bass.Bass()
    kin_d = nc.declare_dram_parameter("kin", [100, KIN_COLS], BF16, isOutput=False)
    out_d = nc.declare_dram_parameter("out", [1, 32], F32, isOutput=True)

    with (
        nc.sbuf_tensor("kin_t", [100, KIN_COLS], BF16) as kin_t,
        nc.sbuf_tensor("out1", [20, 200], BF16) as out1,
        nc.sbuf_tensor("red32", [32, 32], F32) as red32,
        nc.sbuf_tensor("fin32", [32, 32], F32) as fin32,
        nc.psum_tensor("ps1", [20, 200], F32) as ps1,
        nc.psum_tensor("ps2", [20, 200], F32) as ps2,
        nc.psum_tensor("warm", [20, 512], F32) as warm,
        nc.semaphore("in_sem") as in_sem,
        nc.semaphore("pe_sem") as pe_sem,
        nc.semaphore("dve_sem") as dve_sem,
        nc.semaphore("out_sem") as out_sem,
    ):
        # input split across both HWDGE queues (sync + scalar) to halve the
        # descriptor stream per queue
        nc.sync.dma_start(out=kin_t.ap()[0:50, :], in_=kin_d[0:50, :]).then_inc(
            in_sem, 16
        )
        nc.scalar.dma_start(out=kin_t.ap()[50:100, :], in_=kin_d[50:100, :]).then_inc(
            in_sem, 16
        )

        # PE warmup during the DMA wait: read garbage SBUF into a scratch
        # PSUM bank; results never consumed. Keeps the PE pipeline out of
        # its cold p-state for the real matmuls.
        for _ in range(5):
            nc.tensor.matmul(
                warm.ap(),
                kin_t.ap()[0:20, 0:20],
                kin_t.ap()[0:20, 0:512],
                start=True,
                stop=True,
            )

        # stage 1: 4 accumulating matmuls over (y,z) chunks of 100
        nc.tensor.wait_ge(in_sem, 32)
        for c in range(4):
            mm = nc.tensor.matmul(
                ps1.ap(),
                kin_t.ap()[:, 20 * c : 20 * (c + 1)],
                kin_t.ap()[:, 80 + 200 * c : 280 + 200 * c],
                start=(c == 0),
                stop=(c == 3),
            )
        mm.then_inc(pe_sem, 1)

        # PSUM -> SBUF bf16 for stage-2 moving operand (DVE, not Scalar:
        # scalar.copy would trigger a ~2.3us ACT_TABLE_LOAD in the hot path)
        nc.vector.memset(red32.ap(), 3.4e38)
        nc.vector.wait_ge(pe_sem, 1)
        nc.vector.tensor_scalar_add(out1.ap(), ps1.ap(), 0.0).then_inc(dve_sem, 1)

        # stage 2: contract x with Kx, then matmul-accumulate the row mask
        # (identity stationary) so DVE needs no separate mask add
        nc.tensor.wait_ge(dve_sem, 1)
        nc.tensor.matmul(
            ps2.ap(), kin_t.ap()[0:20, 880:900], out1.ap(), start=True, stop=False
        )
        nc.tensor.matmul(
            ps2.ap(),
            kin_t.ap()[0:20, 1100:1120],
            kin_t.ap()[0:20, 900:1100],
            start=False,
            stop=True,
        ).then_inc(pe_sem, 1)

        # masked min over free dim straight from PSUM
        nc.vector.wait_ge(pe_sem, 2)
        nc.vector.tensor_reduce(
            red32.ap()[0:20, 0:1],
            ps2.ap(),
            axis=mybir.AxisListType.X,
            op=mybir.AluOpType.min,
        ).then_inc(dve_sem, 1)

        # 32x32 stream transpose puts the 20 per-x minima on partition 0
        # (same-engine RAW: DVE is pipelined, needs the self-wait)
        nc.vector.wait_ge(dve_sem, 2)
        nc.vector.transpose(fin32.ap(), red32.ap()).then_inc(dve_sem, 1)

        nc.sync.wait_ge(dve_sem, 3)
        nc.sync.dma_start(
            out=out_d[:], in_=fin32.ap()[0:1, 0:32], single_packet=True
        ).then_inc(out_sem, 16)
        nc.sync.wait_ge(out_sem, 16)
    return nc


def _make_kin(rows_mask, cols_mask, half):
    """Build the bf16 kin + fp32 msk inputs for one (problem, half)."""
    k2, kx = _kernels()
    b3 = cols_mask.reshape(D, H, W)
    byz_x = np.ascontiguousarray(b3.transpose(1, 2, 0).reshape(400, D)).astype(NPBF16)
    rows3 = rows_mask.reshape(D, H, W)
    maskbig = np.where(
        rows3[:, 10 * half : 10 * (half + 1), :].reshape(D, 200), 0.0, MASK_BIG
    ).astype(NPBF16)

    kin = np.zeros((100, KIN_COLS), NPBF16)
    for c in range(4):
        kin[:, 20 * c : 20 * (c + 1)] = byz_x[100 * c : 100 * (c + 1)]
        kin[:, 80 + 200 * c : 280 + 200 * c] = k2[
            100 * c : 100 * (c + 1), 200 * half : 200 * (half + 1)
        ]
    kin[0:20, 880:900] = kx
    kin[0:20, 900:1100] = maskbig
    kin[0:20, 1100:1120] = np.eye(20, dtype=NPBF16)
    return kin


def _exact_md2max(rows_mask, cols_mask):
    """Host fallback: exact max-min squared distance (integer grid)."""
    x, y, z = np.meshgrid(np.arange(D), np.arange(H), np.arange(W), indexing="ij")
    coords = np.stack([x, y, z], -1).reshape(V, 3).astype(np.float64)
    rows = coords[rows_mask]
    cols = coords[cols_mask]
    best = 0.0
    for i in range(0, len(rows), 512):
        blk = rows[i : i + 512]
        d2 = ((blk[:, None, :] - cols[None, :, :]) ** 2).sum(-1)
        best = max(best, d2.min(axis=1).max())
    return best


def kernel(predict, target):
    predict = np.asarray(predict)
    target = np.asarray(target)
    n = predict.shape[0]
    im_a = np.round(predict.reshape(n, V)) != 0
    im_b = np.round(target.reshape(n, V)) != 0

    # 2*n directed problems: (rows = one-sided points, cols = other full set)
    probs = []
    for s in range(n):
        ma, mb = im_a[s], im_b[s]
        probs.append((ma & ~mb, mb))  # distA direction
        probs.append((mb & ~ma, ma))  # distB direction
    n_probs = len(probs)
    assert n_probs * 2 == N_CORES, "kernel hardcodes N=2 samples -> 4 problems"

    nc = _build()
    in_maps = []
    for c in range(N_CORES):
        rows_mask, cols_mask = probs[c // 2]
        if not rows_mask.any() or not cols_mask.any():
            # degenerate problem: feed a dummy; host overrides the result
            rows_mask = np.ones(V, bool)
            cols_mask = np.ones(V, bool)
        in_maps.append({"kin": _make_kin(rows_mask, cols_mask, c % 2)})
    results = run_bass_kernel_spmd(nc, in_maps, list(range(N_CORES))).results

    dists = np.empty(n_probs, np.float64)
    for p in range(n_probs):
        rows_mask, cols_mask = probs[p]
        if not rows_mask.any():
            dists[p] = 0.0
            continue
        if not cols_mask.any():
            dists[p] = BIG  # reference: min over empty B stays BIG
            continue
        s_min = min(
            float(np.asarray(results[2 * p]["out"]).reshape(-1)[:20].min()),
            float(np.asarray(results[2 * p + 1]["out"]).reshape(-1)[:20].min()),
        )
        est = -np.log(max(s_min, 1e-45)) / ALPHA if s_min > 0 else np.inf
        if s_min < S_FLOOR or abs(est - np.round(est)) > 0.4:
            md2 = _exact_md2max(rows_mask, cols_mask)  # device range exceeded
        else:
            md2 = float(np.round(est))
        dists[p] = np.sqrt(md2) / 20.0

    haus = np.empty(n, np.float64)
    for s in range(n):
        dist_a, dist_b = dists[2 * s], dists[2 * s + 1]
        ma, mb = im_a[s], im_b[s]
        if (mb & ~ma).any() and not ma.any():
            dist_b = 999.0
        haus[s] = max(dist_a, dist_b)
    return np.float32(haus.mean())


# revision 28
# speedup vs baseline: 3.7498x; 1.0839x over previous
"""Hausdorff distance kernel for Trainium2 (8 NeuronCores).

Reference computes, per sample n (N=2), on a 20^3 voxel grid (V=8000):
  d[i,j]   = Euclidean distance between voxel centers (coords / 20)
  min_to_B = min over j in B of d[i,j]
  distA    = max over i in Aonly of min_to_B   (Aonly = A & ~B)
  (symmetrically distB), haus_n = max(distA, distB); output = mean_n haus_n.

Strategy (separable Gaussian-sum distance field):
  On the integer grid, md2[p] = min_{q in B} |p-q|^2 is an integer. With
  S[p] = sum_{q in B} exp(-a*|p-q|^2) and a=10, we get
      -ln(S[p])/a = md2[p] - ln(n0 + eps)/a,   n0 = #minimizers <= 24,
  so round(-ln(S)/a) recovers md2 EXACTLY while S stays in fp32 range
  (md2 <= 8; larger values underflow detectably -> exact host fallback,
  which cannot trigger unless the Hausdorff distance exceeds sqrt(8)
  voxels).  exp(-a*|p-q|^2) factors over axes, so S = (Kx x Ky x Kz) * B
  is computed with 5 small matmuls per (sample, direction):
    stage 1:  out1[x,(y',z')] = sum_{(y,z)} B[(y,z),x] * K2[(y,z),(y',z')]
              (4 PSUM-accumulated matmuls, contraction 400 split 4x100)
    stage 2:  S[x',(y',z')]   = sum_x Kx[x,x'] * out1[x,(y',z')]
  then min over the Aonly mask via one DVE tensor_tensor_reduce and a
  GpSimd partition-min to a single scalar (1-packet output DMA keeps the
  DMA-completion tail short).  8 cores = 4 (sample,direction) problems
  x 2 halves of the (y',z') output space.
"""

import sys
import functools

import numpy as np

for _p in ("/opt/trn_rl_repo",):
    if _p not in sys.path:
        sys.path.insert(0, _p)

import ml_dtypes  # noqa: E402
from concourse import bass, mybir  # noqa: E402
from concourse.bass_utils import run_bass_kernel_spmd  # noqa: E402

D = H = W = 20
V = D * H * W
N_CORES = 8
BIG = 1e9
ALPHA = 10.0
S_FLOOR = 1e-36  # S below this => md2 >= 9 possible => exact fallback
MASK_BIG = 3.0e38
F32 = mybir.dt.float32
BF16 = mybir.dt.bfloat16
NPBF16 = ml_dtypes.bfloat16

# kin layout (bf16, [100, 880]); only the 3 y-groups within |dy|<=2 of the
# core's (y',z') half contribute (the 4th K2 chunk is ~exp(-360), i.e. 0):
#   chunk c in {0,1,2}: cols [220c, 220c+220): B-mask [:, 0:20), K2 [:, 20:220)
#   [0:40, 660:680)  stacked stationary [Kx ; I] for fused stage2+mask matmul
#   [0:20, 680:880)  row mask as 0.0 / MASK_BIG for this (y',z') half
#                    (re-DMA'd into partitions 20:40 of the rhs2 tensor)
KIN_COLS = 880


@functools.lru_cache(maxsize=None)
def _kernels():
    yy, zz = np.meshgrid(np.arange(H), np.arange(W), indexing="ij")
    yz = np.stack([yy, zz], -1).reshape(400, 2).astype(np.float64)
    k2 = np.exp(-ALPHA * ((yz[:, None, :] - yz[None, :, :]) ** 2).sum(-1))
    kx = np.exp(-ALPHA * (np.subtract.outer(np.arange(D), np.arange(D)) ** 2.0))
    return k2.astype(NPBF16), kx.astype(NPBF16)


@functools.lru_cache(maxsize=None)
def _build():
    """Per-core program: S-field for one (problem, half) -> masked min scalar."""
    nc = bass.Bass()
    kin_d = nc.declare_dram_parameter("kin", [100, KIN_COLS], BF16, isOutput=False)
    out_d = nc.declare_dram_parameter("out", [1, 32], F32, isOutput=True)

    with (
        nc.sbuf_tensor("kin_t", [100, KIN_COLS], BF16) as kin_t,
        nc.sbuf_tensor("rhs2", [40, 200], BF16) as rhs2,
        nc.sbuf_tensor("red32", [32, 32], F32) as red32,
        nc.sbuf_tensor("fin32", [32, 32], F32) as fin32,
        nc.psum_tensor("ps1", [20, 200], F32) as ps1,
        nc.psum_tensor("ps2", [20, 200], F32) as ps2,
        nc.psum_tensor("warm", [20, 512], F32) as warm,
        nc.semaphore("ina_sem") as ina_sem,
        nc.semaphore("inb_sem") as inb_sem,
        nc.semaphore("inc_sem") as inc_sem,
        nc.semaphore("pe_sem") as pe_sem,
        nc.semaphore("dve_sem") as dve_sem,
        nc.semaphore("out_sem") as out_sem,
    ):
        # inputs pipelined across both HWDGE queues: chunks 0-1 on sync,
        # chunk 2 + stationary + mask on scalar, mask replica on sync
        nc.sync.dma_start(out=kin_t.ap()[:, 0:440], in_=kin_d[:, 0:440]).then_inc(
            ina_sem, 16
        )
        nc.scalar.dma_start(out=kin_t.ap()[:, 440:880], in_=kin_d[:, 440:880]).then_inc(
            inb_sem, 16
        )
        nc.sync.dma_start(out=rhs2.ap()[20:40, :], in_=kin_d[0:20, 680:880]).then_inc(
            inc_sem, 16
        )

        # PE warmup during the DMA wait: read garbage SBUF into a scratch
        # PSUM bank; results never consumed. Keeps the PE pipeline out of
        # its cold p-state for the real matmuls.
        for _ in range(4):
            nc.tensor.matmul(
                warm.ap(),
                kin_t.ap()[0:20, 0:20],
                kin_t.ap()[0:20, 0:512],
                start=True,
                stop=True,
            )

        # stage 1: 3 accumulating matmuls over the relevant (y,z) chunks
        nc.tensor.wait_ge(ina_sem, 16)
        for c in range(2):
            nc.tensor.matmul(
                ps1.ap(),
                kin_t.ap()[:, 220 * c : 220 * c + 20],
                kin_t.ap()[:, 220 * c + 20 : 220 * (c + 1)],
                start=(c == 0),
                stop=False,
            )
        nc.tensor.wait_ge(inb_sem, 16)
        nc.tensor.matmul(
            ps1.ap(), kin_t.ap()[:, 440:460], kin_t.ap()[:, 460:660],
            start=False, stop=True,
        ).then_inc(pe_sem, 1)

        # PSUM -> SBUF bf16 into rhs2[0:20] (DVE, not Scalar: scalar.copy
        # would trigger a ~2.3us ACT_TABLE_LOAD in the hot path)
        nc.vector.memset(red32.ap(), 3.4e38)
        nc.vector.wait_ge(pe_sem, 1)
        nc.vector.tensor_scalar_add(rhs2.ap()[0:20, :], ps1.ap(), 0.0).then_inc(
            dve_sem, 1
        )

        # fused stage 2: [Kx ; I]^T @ [out1 ; mask] contracts x AND adds the
        # row mask in a single matmul
        nc.tensor.wait_ge(inc_sem, 16)
        nc.tensor.wait_ge(dve_sem, 1)
        nc.tensor.matmul(
            ps2.ap(), kin_t.ap()[0:40, 660:680], rhs2.ap(), start=True, stop=True
        ).then_inc(pe_sem, 1)

        # masked min over free dim straight from PSUM
        nc.vector.wait_ge(pe_sem, 2)
        nc.vector.tensor_reduce(
            red32.ap()[0:20, 0:1],
            ps2.ap(),
            axis=mybir.AxisListType.X,
            op=mybir.AluOpType.min,
        ).then_inc(dve_sem, 1)

        # 32x32 stream transpose puts the 20 per-x minima on partition 0
        # (same-engine RAW: DVE is pipelined, needs the self-wait)
        nc.vector.wait_ge(dve_sem, 2)
        nc.vector.transpose(fin32.ap(), red32.ap()).then_inc(dve_sem, 1)

        nc.sync.wait_ge(dve_sem, 3)
        nc.sync.dma_start(
            out=out_d[:], in_=fin32.ap()[0:1, 0:32], single_packet=True
        ).then_inc(out_sem, 16)
        nc.sync.wait_ge(out_sem, 16)
    return nc


def _make_kin(rows_mask, cols_mask, half):
    """Build the [100, 880] bf16 input for one (problem, half)."""
    k2, kx = _kernels()
    b3 = cols_mask.reshape(D, H, W)
    byz_x = np.ascontiguousarray(b3.transpose(1, 2, 0).reshape(400, D)).astype(NPBF16)
    rows3 = rows_mask.reshape(D, H, W)
    maskbig = np.where(
        rows3[:, 10 * half : 10 * (half + 1), :].reshape(D, 200), 0.0, MASK_BIG
    ).astype(NPBF16)

    kin = np.zeros((100, KIN_COLS), NPBF16)
    # y-groups of 5 within |dy|<=2 of this half's y' range; the dropped
    # group's K2 entries are <= exp(-10*9) ~ 0 in bf16
    groups = (0, 1, 2) if half == 0 else (1, 2, 3)
    for i, g in enumerate(groups):
        kin[:, 220 * i : 220 * i + 20] = byz_x[100 * g : 100 * (g + 1)]
        kin[:, 220 * i + 20 : 220 * (i + 1)] = k2[
            100 * g : 100 * (g + 1), 200 * half : 200 * (half + 1)
        ]
    kin[0:20, 660:680] = kx
    kin[20:40, 660:680] = np.eye(20, dtype=NPBF16)
    kin[0:20, 680:880] = maskbig
    return kin


def _exact_md2max(rows_mask, cols_mask):
    """Host fallback: exact max-min squared distance (integer grid)."""
    x, y, z = np.meshgrid(np.arange(D), np.arange(H), np.arange(W), indexing="ij")
    coords = np.stack([x, y, z], -1).reshape(V, 3).astype(np.float64)
    rows = coords[rows_mask]
    cols = coords[cols_mask]
    best = 0.0
    for i in range(0, len(rows), 512):
        blk = rows[i : i + 512]
        d2 = ((blk[:, None, :] - cols[None, :, :]) ** 2).sum(-1)
        best = max(best, d2.min(axis=1).max())
    return best


def kernel(predict, target):
    predict = np.asarray(predict)
    target = np.asarray(target)
    n = predict.shape[0]
    im_a = np.round(predict.reshape(n, V)) != 0
    im_b = np.round(target.reshape(n, V)) != 0

    # 2*n directed problems: (rows = one-sided points, cols = other full set)
    probs = []
    for s in range(n):
        ma, mb = im_a[s], im_b[s]
        probs.append((ma & ~mb, mb))  # distA direction
        probs.append((mb & ~ma, ma))  # distB direction
    n_probs = len(probs)
    assert n_probs * 2 == N_CORES, "kernel hardcodes N=2 samples -> 4 problems"

    nc = _build()
    in_maps = []
    for c in range(N_CORES):
        rows_mask, cols_mask = probs[c // 2]
        if not rows_mask.any() or not cols_mask.any():
            # degenerate problem: feed a dummy; host overrides the result
            rows_mask = np.ones(V, bool)
            cols_mask = np.ones(V, bool)
        in_maps.append({"kin": _make_kin(rows_mask, cols_mask, c % 2)})
    results = run_bass_kernel_spmd(nc, in_maps, list(range(N_CORES))).results

    dists = np.empty(n_probs, np.float64)
    for p in range(n_probs):
        rows_mask, cols_mask = probs[p]
        if not rows_mask.any():
            dists[p] = 0.0
            continue
        if not cols_mask.any():
            dists[p] = BIG  # reference: min over empty B stays BIG
            continue
        s_min = min(
            float(np.asarray(results[2 * p]["out"]).reshape(-1)[:20].min()),
            float(np.asarray(results[2 * p + 1]["out"]).reshape(-1)[:20].min()),
        )
        est = -np.log(max(s_min, 1e-45)) / ALPHA if s_min > 0 else np.inf
        if s_min < S_FLOOR or abs(est - np.round(est)) > 0.4:
            md2 = _exact_md2max(rows_mask, cols_mask)  # device range exceeded
        else:
            md2 = float(np.round(est))
        dists[p] = np.sqrt(md2) / 20.0

    haus = np.empty(n, np.float64)
    for s in range(n):
        dist_a, dist_b = dists[2 * s], dists[2 * s + 1]
        ma, mb = im_a[s], im_b[s]
        if (mb & ~ma).any() and not ma.any():
            dist_b = 999.0
        haus[s] = max(dist_a, dist_b)
    return np.float32(haus.mean())
